# revision 19
# baseline (speedup 1.0000x reference)
"""Trainium2 Bass kernel for nn_Attention_37598143709539.

Dense transformer attention with a 1x1-conv relative positional bias:
  qkv = x @ Wqkv ; per-head scores = q k^T * scale + conv1x1(centroid_delta)
  out = softmax(scores) @ v ; final = concat-heads @ Wout + bout

Distribution: pure data-parallel over (batch, query-half) -> 8 cores; core
cid handles batch cid//2, query rows [cid%2*1024, +1024).  Keys/values and
the softmax run over the full 2048-key axis locally, so no collectives are
needed; the host concatenates the 8 output shards.

On-core layout: everything stays "feature-major" (transposed) so matmuls
chain without on-chip transposes of activations:
  scoresT[j, i] = k_h^T q_h      (key token j on partitions, query i free)
  p = exp(scoresT + biasT)       (max-free softmax: logits are O(13) for
                                  this data, safely inside fp32 exp range)
  attn-outT accumulated with lhsT = [v_h | ones]: the ones column yields
  the softmax normalizer for free, and outT chains directly into Wout.

The conv bias (3 channels x rel_w) is applied per [128,1024] scores plane
by one of two statically-interleaved routes that balance the engines
(GPSIMD tensor ops measured ~15x slower than modeled and starve DVE via
the shared SBUF port, so they are not used):
  A: PE accumulates w[h,c]*I identity-matmuls into the scores PSUM (exp
     then reads PSUM directly).  The scaled identities are built per-head
     on the ACT engine.
  B: DVE scalar_tensor_tensor chain (3 MACs) from PSUM into an SBUF f16
     pair tile; the exps of two adjacent B planes batch into a single
     [128, 2048] ACTIVATE to amortize ACT per-op overhead.
A ~10us dummy-matmul warmup spin at kernel start keeps the PE HAM clock
gate at full rate through the DMA-bound qkv phase.
"""

from contextlib import ExitStack

import numpy as np

import concourse.bass as bass
import concourse.mybir as mybir
import concourse.tile as tile
from concourse import bacc
from concourse.masks import make_identity

B, N, D = 4, 2048, 512
HEADS, DH = 8, 64
SCALE = DH ** -0.5
P = 128
IH = N // 2            # query rows handled per core
NCORES = 8
BF = mybir.dt.bfloat16
F16 = mybir.dt.float16
F32 = mybir.dt.float32
MULT = mybir.AluOpType.mult
ADD = mybir.AluOpType.add
EXP = mybir.ActivationFunctionType.Exp

# per-head route pattern over the 16 key-tile planes (B's in adjacent
# pairs so their exps batch into one ACTIVATE)
ROUTE16 = "ABBABBAABBABBAAA"
N_WARMUP = 120


def build_bass():
    nc = bacc.Bacc(None)
    x_d = nc.declare_dram_parameter("x", [N, D], F16, isOutput=False)
    xq_d = nc.declare_dram_parameter("xq", [IH, D], F16, isOutput=False)
    cd_d = nc.declare_dram_parameter("cd", [3, N, IH], F16, isOutput=False)  # pre-transposed [c, j, i]
    wqkv_d = nc.declare_dram_parameter("wqkv", [D, 3 * D], F16, isOutput=False)
    wout_d = nc.declare_dram_parameter("wout", [D, D], F16, isOutput=False)
    bout_d = nc.declare_dram_parameter("bout", [D], F32, isOutput=False)
    relw_d = nc.declare_dram_parameter("relw", [HEADS, 3], F32, isOutput=False)
    relb_d = nc.declare_dram_parameter("relb", [HEADS], F32, isOutput=False)
    out_d = nc.declare_dram_parameter("out", [IH, D], F32, isOutput=True)

    def bcast(ap, parts=P):
        # replicate a DRAM AP across all partitions (step-0 partition dim)
        return bass.AP(tensor=ap.tensor, offset=ap.offset, ap=[[0, parts], *ap.ap])

    with ExitStack() as ctx:
        tc = ctx.enter_context(tile.TileContext(nc))
        singles = ctx.enter_context(tc.tile_pool(name="singles", bufs=1))
        cdtp = ctx.enter_context(tc.tile_pool(name="cdtp", bufs=1))
        drp = ctx.enter_context(tc.tile_pool(name="drp", bufs=4, space="DRAM"))
        psc = ctx.enter_context(tc.tile_pool(name="psc", bufs=3, space="PSUM"))
        pat = ctx.enter_context(tc.tile_pool(name="pat", bufs=1, space="PSUM"))

        # ---- constants (SWDGE ring, keeps the SP/xbar ring free) ----
        relw_sb = singles.tile([P, HEADS, 3], F32)
        nc.gpsimd.dma_start(out=relw_sb, in_=bcast(relw_d[:, :]))
        relb_sb = singles.tile([P, HEADS], F32)
        nc.gpsimd.dma_start(out=relb_sb, in_=bcast(relb_d[:]))
        bout_sb = singles.tile([P, D], F32)
        nc.gpsimd.dma_start(out=bout_sb, in_=bcast(bout_d[:]))
        ident = singles.tile([P, P], F16)
        make_identity(nc, ident)
        wout_sb = singles.tile([P, 4, D], F16)
        nc.gpsimd.dma_start(out=wout_sb, in_=wout_d.rearrange("(dc p) f -> p dc f", p=P))
        wqp = tc.alloc_tile_pool(name="wqp", bufs=1)
        wqkv_sb = wqp.tile([P, 4, 3 * D], F16, tag="wqkv_sb")
        nc.gpsimd.dma_start(out=wqkv_sb, in_=wqkv_d.rearrange("(dc p) f -> p dc f", p=P))

        # ---- PE warmup spin: dummy matmuls keep the HAM clock-gate hot
        # through the DMA-bound lead-in (no deps beyond the identity tile)
        wu = psc.tile([P, IH], F32, tag="ps_s")
        for _ in range(N_WARMUP):
            nc.tensor.matmul(wu[:, 0:P], lhsT=ident, rhs=ident,
                             start=True, stop=True, skip_group_check=True)

        # ---- qkv projection (fp16, f32 PSUM accumulate) ----
        # x/xq enter feature-major via quartered DRAM->SBUF xbar transposes
        # so the first matmuls start after the first quarter lands.
        # SCALE is pre-folded into the q-columns of Wqkv on the host.
        qT = singles.tile([P, 4, IH], F16)        # [f%128, fo, i]  (scaled)
        kT = singles.tile([P, 4, N], F16)         # [f%128, fo, j]
        v_sb = singles.tile([P, 16, HEADS, DH + 1], BF)  # [j%128, jt, h, dh | 1s]
        xqpool = tc.alloc_tile_pool(name="xqpool", bufs=1)
        xqT = xqpool.tile([P, 2, 4, 512], F16, tag="xqT")   # [d%128, tq, dc, t%512]
        for tq in range(2):
            nc.sync.dma_start_transpose(out=xqT[:, tq, :, :],
                                        in_=xq_d[tq * 512:(tq + 1) * 512, :])
        for t2 in range(IH // 512):
            for fo in range(4):
                ps = psc.tile([P, IH], F32, tag="ps_s")
                for dc in range(4):
                    nc.tensor.matmul(ps[:, 0:512],
                                     lhsT=wqkv_sb[:, dc, fo * P:(fo + 1) * P],
                                     rhs=xqT[:, t2, dc, :],
                                     start=(dc == 0), stop=(dc == 3))
                nc.any.tensor_copy(out=qT[:, fo, t2 * 512:(t2 + 1) * 512],
                                   in_=ps[:, 0:512])
        xqpool.release()
        xtpool = tc.alloc_tile_pool(name="xtpool", bufs=1)
        xT = xtpool.tile([P, 4, 4, 512], F16, tag="xT")     # [d%128, tq, dc, t%512]
        for tq in range(4):
            nc.sync.dma_start_transpose(out=xT[:, tq, :, :],
                                        in_=x_d[tq * 512:(tq + 1) * 512, :])
        # prefetch centroid-delta (host pre-transposed): plain SWDGE loads on
        # the GpSimd ring, quartered along keys so early heads start sooner
        cdts = []
        for c in range(3):
            # cdt[j%128, jt, i] = cdT[c, jt*128 + j%128, i]
            cdt = cdtp.tile([P, 16, IH], F16, tag=f"cdt{c}")
            cdts.append(cdt)
        for jq in range(4):
            for c in range(3):
                nc.gpsimd.dma_start(
                    out=cdts[c][:, jq * 4:(jq + 1) * 4, :],
                    in_=cd_d[c, jq * 512:(jq + 1) * 512, :].rearrange(
                        "(jt p) i -> p jt i", p=P))
        for t4 in range(N // 512):
            for fo in range(4):
                ps = psc.tile([P, IH], F32, tag="ps_s")
                for dc in range(4):
                    nc.tensor.matmul(ps[:, 0:512],
                                     lhsT=wqkv_sb[:, dc, D + fo * P:D + (fo + 1) * P],
                                     rhs=xT[:, t4, dc, :],
                                     start=(dc == 0), stop=(dc == 3))
                nc.any.tensor_copy(out=kT[:, fo, t4 * 512:(t4 + 1) * 512], in_=ps[:, 0:512])
        for tt in range(16):
            ps = psc.tile([P, IH], F32, tag="ps_s")
            for dc in range(4):
                nc.tensor.matmul(ps[:, 0:512],
                                 lhsT=xT[:, tt // 4, dc, (tt % 4) * P:(tt % 4 + 1) * P],
                                 rhs=wqkv_sb[:, dc, 2 * D:3 * D],
                                 start=(dc == 0), stop=(dc == 3))
            nc.any.tensor_copy(out=v_sb[:, tt, :, 0:DH],
                               in_=ps[:, 0:512].rearrange("p (h d) -> p h d", h=HEADS))
        nc.vector.memset(v_sb[:, :, :, DH:DH + 1], 1.0)
        xtpool.release()
        wqp.release()

        # ---- attention: head-serial, full-width (1024) planes ----
        sbt = ctx.enter_context(tc.tile_pool(name="sbt", bufs=2))
        ptp = ctx.enter_context(tc.tile_pool(name="ptp", bufs=2))
        idp = ctx.enter_context(tc.tile_pool(name="idp", bufs=2))
        normp = ctx.enter_context(tc.tile_pool(name="normp", bufs=2))
        bcp = ctx.enter_context(tc.tile_pool(name="bcp", bufs=1))
        outp = ctx.enter_context(tc.tile_pool(name="outp", bufs=2))
        outT = singles.tile([P, 4, IH], F16)      # [f%128, fo, i]

        for h in range(HEADS):
            fo, hp = h // 2, (h % 2) * 64
            # scaled identities for route A, built on ACT (idle-ish here)
            idh = idp.tile([P, 3, P], F16, tag="idh")
            for c in range(3):
                nc.scalar.mul(idh[:, c, :], ident, relw_sb[:, h, c:c + 1])
            po = pat.tile([P, IH], F32, tag="po")
            bpend = None     # pending half-filled B exp pair: (jt, b3, pTp)
            for jt in range(16):
                r = ROUTE16[jt]
                ps = psc.tile([P, IH], F32, tag="ps_s")
                pv = []      # (jt, pT-slice) planes ready for the pv matmul
                if r == 'A':
                    # bias first: it depends only on cd + idh, so the PE can
                    # run it while q/k are still being produced (or while
                    # waiting on the exp of older planes)
                    for c in range(3):
                        for half in range(2):
                            sl5 = slice(half * 512, (half + 1) * 512)
                            nc.tensor.matmul(ps[:, sl5], lhsT=idh[:, c, :],
                                             rhs=cdts[c][:, jt, sl5],
                                             start=(c == 0), stop=False,
                                             skip_group_check=True)
                    for half in range(2):
                        sl5 = slice(half * 512, (half + 1) * 512)
                        nc.tensor.matmul(ps[:, sl5],
                                         lhsT=kT[hp:hp + 64, fo, jt * P:(jt + 1) * P],
                                         rhs=qT[hp:hp + 64, fo, sl5],
                                         start=False, stop=(half == 1),
                                         skip_group_check=True)
                    pT = ptp.tile([P, IH], BF, tag="pT")
                    nc.scalar.activation(out=pT, in_=ps, func=EXP,
                                         bias=relb_sb[:, h:h + 1], scale=1.0)
                    pv.append((jt, pT[:, :]))
                else:
                    for half in range(2):
                        sl5 = slice(half * 512, (half + 1) * 512)
                        nc.tensor.matmul(ps[:, sl5],
                                         lhsT=kT[hp:hp + 64, fo, jt * P:(jt + 1) * P],
                                         rhs=qT[hp:hp + 64, fo, sl5],
                                         start=True, stop=True, skip_group_check=True)
                    if bpend is None:
                        b3 = sbt.tile([P, 2, IH], F16, tag="b3")
                        pTp = ptp.tile([P, 2, IH], BF, tag="pTp")
                        side = 0
                    else:
                        _, b3, pTp = bpend
                        side = 1
                    t1 = sbt.tile([P, IH], F16, tag="t1")
                    nc.vector.scalar_tensor_tensor(
                        out=t1, in0=cdts[0][:, jt, :], scalar=relw_sb[:, h, 0:1],
                        in1=ps, op0=MULT, op1=ADD)
                    nc.vector.scalar_tensor_tensor(
                        out=b3[:, side, :], in0=cdts[1][:, jt, :],
                        scalar=relw_sb[:, h, 1:2], in1=t1, op0=MULT, op1=ADD)
                    nc.vector.scalar_tensor_tensor(
                        out=b3[:, side, :], in0=cdts[2][:, jt, :],
                        scalar=relw_sb[:, h, 2:3], in1=b3[:, side, :],
                        op0=MULT, op1=ADD)
                    if side == 0:
                        bpend = (jt, b3, pTp)
                    else:
                        jt0, b3, pTp = bpend
                        bpend = None
                        nc.scalar.activation(out=pTp, in_=b3, func=EXP,
                                             bias=relb_sb[:, h:h + 1], scale=1.0)
                        pv.append((jt0, pTp[:, 0, :]))
                        pv.append((jt, pTp[:, 1, :]))
                for jtp, pTs in pv:
                    for half in range(2):
                        sl5 = slice(half * 512, (half + 1) * 512)
                        nc.tensor.matmul(po[0:DH + 1, sl5], lhsT=v_sb[:, jtp, h, :],
                                         rhs=pTs[:, sl5], start=(jtp == 0),
                                         stop=(jtp == 15), skip_group_check=True)
            # evict po immediately (ACT) so the next head's accumulation can
            # start while the normalizer DMA round-trip is in flight
            o_sb = normp.tile([P, IH], F32, tag="o_sb")
            nc.scalar.copy(out=o_sb[0:DH + 1, :], in_=po[0:DH + 1, :])
            dr = drp.tile([IH], F32, tag="dr")
            nc.sync.dma_start(out=dr[:], in_=o_sb[64:65, :])
            sl = normp.tile([P, 8], F32, tag="sl")
            nc.sync.dma_start(out=sl, in_=dr.rearrange("(p c) -> p c", p=P))
            rs = normp.tile([P, 8], F32, tag="rs")
            nc.vector.reciprocal(out=rs, in_=sl)
            dr2 = drp.tile([IH], F32, tag="dr2")
            nc.sync.dma_start(out=dr2.rearrange("(p c) -> p c", p=P), in_=rs)
            bc = bcp.tile([P, IH], F32, tag="bc")
            nc.sync.dma_start(out=bc[0:64, :], in_=bcast(dr2[:], parts=64))
            if hp == 0:
                nc.vector.tensor_mul(outT[0:64, fo, :], o_sb[0:64, :], bc[0:64, :])
            else:
                tmp = bcp.tile([P, IH], F16, tag="tmp")
                nc.vector.tensor_mul(tmp[0:64, :], o_sb[0:64, :], bc[0:64, :])
                nc.sync.dma_start(out=outT[64:128, fo, :], in_=tmp[0:64, :])

        # ---- output projection ----
        for tt in range(IH // P):
            ps = psc.tile([P, IH], F32, tag="ps_s")
            for fo in range(4):
                nc.tensor.matmul(ps[:, 0:512], lhsT=outT[:, fo, tt * P:(tt + 1) * P],
                                 rhs=wout_sb[:, fo, :], start=(fo == 0), stop=(fo == 3))
            osb = outp.tile([P, D], F32, tag="osb")
            nc.vector.scalar_tensor_tensor(out=osb, in0=ps[:, 0:512], scalar=1.0,
                                           in1=bout_sb, op0=MULT, op1=ADD)
            nc.sync.dma_start(out=out_d[tt * P:(tt + 1) * P, :], in_=osb)

    nc.finalize()
    return nc


_CACHE = {}


def _run(in_maps, trace=False, **kw):
    from concourse.bass_utils import run_bass_kernel_spmd
    nc = _CACHE.get('nc')
    if nc is None:
        nc = build_bass()
        _CACHE['nc'] = nc
    return run_bass_kernel_spmd(nc, in_maps, list(range(NCORES)), trace=trace, **kw)


def make_in_maps(x, centroid_delta, Wqkv, Wout, bout, rel_w, rel_b):
    f32 = lambda a: np.ascontiguousarray(np.asarray(a, dtype=np.float32))
    f16 = lambda a: np.ascontiguousarray(np.asarray(a, dtype=np.float32).astype(np.float16))
    Wqkv = np.asarray(Wqkv, dtype=np.float32).copy()
    Wqkv[:, :D] *= SCALE          # fold the attention scale into the q columns
    x = f16(x)
    centroid_delta = f16(centroid_delta)
    Wqkv = f16(Wqkv)
    Wout = f16(Wout)
    bout = f32(bout)
    rel_w = f32(rel_w)
    rel_b = f32(rel_b)
    in_maps = []
    for cid in range(NCORES):
        b, ihf = cid // 2, cid % 2
        sl = slice(ihf * IH, (ihf + 1) * IH)
        in_maps.append({
            "x": x[b],
            "xq": np.ascontiguousarray(x[b, sl]),
            "cd": np.ascontiguousarray(centroid_delta[b, :, sl, :].transpose(0, 2, 1)),
            "wqkv": Wqkv,
            "wout": Wout,
            "bout": bout,
            "relw": rel_w,
            "relb": rel_b,
        })
    return in_maps


def assemble(results):
    out = np.empty((B, N, D), dtype=np.float32)
    for cid in range(NCORES):
        b, ihf = cid // 2, cid % 2
        out[b, ihf * IH:(ihf + 1) * IH, :] = results[cid]["out"]
    return out


def kernel(x, centroid_delta, Wqkv, Wout, bout, rel_w, rel_b):
    in_maps = make_in_maps(x, centroid_delta, Wqkv, Wout, bout, rel_w, rel_b)
    res = _run(in_maps, trace=False)
    return assemble(res.results)


# revision 27
# speedup vs baseline: 1.1920x; 1.1920x over previous
"""Trainium2 Bass kernel for nn_Attention_37598143709539.

Dense transformer attention with a 1x1-conv relative positional bias:
  qkv = x @ Wqkv ; per-head scores = q k^T * scale + conv1x1(centroid_delta)
  out = softmax(scores) @ v ; final = concat-heads @ Wout + bout

Distribution: pure data-parallel over (batch, query-half) -> 8 cores; core
cid handles batch cid//2, query rows [cid%2*1024, +1024).  Keys/values and
the softmax run over the full 2048-key axis locally, so no collectives are
needed; the host concatenates the 8 output shards.

On-core layout: everything stays "feature-major" (transposed) so matmuls
chain without on-chip transposes of activations:
  scoresT[j, i] = k_h^T q_h      (key token j on partitions, query i free)
  p = exp(scoresT + biasT)       (max-free softmax: logits are O(13) for
                                  this data, safely inside fp32 exp range)
  attn-outT accumulated with lhsT = [v_h | ones]: the ones column yields
  the softmax normalizer for free, and outT chains directly into Wout.

The conv bias (3 channels x rel_w) is applied per [128,1024] scores plane
by one of two statically-interleaved routes that balance the engines
(GPSIMD tensor ops measured ~15x slower than modeled and starve DVE via
the shared SBUF port, so they are not used):
  A: PE accumulates w[h,c]*I identity-matmuls into the scores PSUM (exp
     then reads PSUM directly).  The scaled identities are built per-head
     on the ACT engine.
  B: DVE scalar_tensor_tensor chain (3 MACs) from PSUM into an SBUF f16
     pair tile; the exps of two adjacent B planes batch into a single
     [128, 2048] ACTIVATE to amortize ACT per-op overhead.
A ~10us dummy-matmul warmup spin at kernel start keeps the PE HAM clock
gate at full rate through the DMA-bound qkv phase.
"""

from contextlib import ExitStack

import numpy as np

import concourse.bass as bass
import concourse.mybir as mybir
import concourse.tile as tile
from concourse import bacc
from concourse.masks import make_identity

B, N, D = 4, 2048, 512
HEADS, DH = 8, 64
SCALE = DH ** -0.5
P = 128
IH = N // 2            # query rows handled per core
NCORES = 8
BF = mybir.dt.bfloat16
F16 = mybir.dt.float16
F32 = mybir.dt.float32
MULT = mybir.AluOpType.mult
ADD = mybir.AluOpType.add
EXP = mybir.ActivationFunctionType.Exp

# per-head route pattern over the 16 key-tile planes (B's in adjacent
# pairs so their exps batch into one ACTIVATE)
ROUTE16 = "ABBAABBAABBAAAAA"
N_WARMUP = 120
HEAD_ORDER = [1, 0, 3, 2, 5, 4, 7, 6]   # last head has hp==0: its outT
                                         # write needs no partition-shift DMA


def build_bass():
    nc = bacc.Bacc(None)
    x_d = nc.declare_dram_parameter("x", [N, D], F16, isOutput=False)
    xq_d = nc.declare_dram_parameter("xq", [IH, D], F16, isOutput=False)
    cd_d = nc.declare_dram_parameter("cd", [3, N, IH], F16, isOutput=False)  # pre-transposed [c, j, i]
    wqkv_d = nc.declare_dram_parameter("wqkv", [D, 3 * D], F16, isOutput=False)
    wout_d = nc.declare_dram_parameter("wout", [D, D], F16, isOutput=False)
    bout_d = nc.declare_dram_parameter("bout", [D], F32, isOutput=False)
    relw_d = nc.declare_dram_parameter("relw", [HEADS, 3], F32, isOutput=False)
    relb_d = nc.declare_dram_parameter("relb", [HEADS], F32, isOutput=False)
    out_d = nc.declare_dram_parameter("out", [IH, D], F32, isOutput=True)

    def bcast(ap, parts=P):
        # replicate a DRAM AP across all partitions (step-0 partition dim)
        return bass.AP(tensor=ap.tensor, offset=ap.offset, ap=[[0, parts], *ap.ap])

    with ExitStack() as ctx:
        tc = ctx.enter_context(tile.TileContext(nc))
        singles = ctx.enter_context(tc.tile_pool(name="singles", bufs=1))
        cdtp = ctx.enter_context(tc.tile_pool(name="cdtp", bufs=1))
        drp = ctx.enter_context(tc.tile_pool(name="drp", bufs=4, space="DRAM"))
        psc = ctx.enter_context(tc.tile_pool(name="psc", bufs=3, space="PSUM"))
        pat = ctx.enter_context(tc.tile_pool(name="pat", bufs=1, space="PSUM"))

        # ---- identity + warmup first, before anything queues on gpsimd,
        # so the PE spin starts immediately and keeps the HAM clock hot
        # through the DMA-bound lead-in
        ident = singles.tile([P, P], F16)
        make_identity(nc, ident)
        wu = psc.tile([P, IH], F32, tag="ps_s")
        for _ in range(N_WARMUP):
            nc.tensor.matmul(wu[:, 0:P], lhsT=ident, rhs=ident,
                             start=True, stop=True, skip_group_check=True)

        # ---- constants (SWDGE ring, keeps the SP/xbar ring free) ----
        relw_sb = singles.tile([P, HEADS, 3], F32)
        nc.gpsimd.dma_start(out=relw_sb, in_=bcast(relw_d[:, :]))
        relb_sb = singles.tile([P, HEADS], F32)
        nc.gpsimd.dma_start(out=relb_sb, in_=bcast(relb_d[:]))
        bout_sb = singles.tile([P, D], F32)
        nc.gpsimd.dma_start(out=bout_sb, in_=bcast(bout_d[:]))
        wout_sb = singles.tile([P, 4, D], F16)
        nc.gpsimd.dma_start(out=wout_sb, in_=wout_d.rearrange("(dc p) f -> p dc f", p=P))
        wqp = tc.alloc_tile_pool(name="wqp", bufs=1)
        wqkv_sb = wqp.tile([P, 4, 3 * D], F16, tag="wqkv_sb")
        nc.gpsimd.dma_start(out=wqkv_sb, in_=wqkv_d.rearrange("(dc p) f -> p dc f", p=P))

        # ---- qkv projection (fp16, f32 PSUM accumulate) ----
        # x/xq enter feature-major via quartered DRAM->SBUF xbar transposes
        # so the first matmuls start after the first quarter lands.
        # SCALE is pre-folded into the q-columns of Wqkv on the host.
        qT = singles.tile([P, 4, IH], F16)        # [f%128, fo, i]  (scaled)
        kT = singles.tile([P, 4, N], F16)         # [f%128, fo, j]
        v_sb = singles.tile([P, 16, HEADS, DH + 1], BF)  # [j%128, jt, h, dh | 1s]
        xqpool = tc.alloc_tile_pool(name="xqpool", bufs=1)
        xqT = xqpool.tile([P, 2, 4, 512], F16, tag="xqT")   # [d%128, tq, dc, t%512]
        xtpool = tc.alloc_tile_pool(name="xtpool", bufs=1)
        xT = xtpool.tile([P, 4, 4, 512], F16, tag="xT")     # [d%128, tq, dc, t%512]
        for tq in range(2):
            nc.sync.dma_start_transpose(out=xqT[:, tq, :, :],
                                        in_=xq_d[tq * 512:(tq + 1) * 512, :])
        for tq in range(4):
            nc.sync.dma_start_transpose(out=xT[:, tq, :, :],
                                        in_=x_d[tq * 512:(tq + 1) * 512, :])
        # prefetch centroid-delta (host pre-transposed): plain SWDGE loads on
        # the GpSimd ring, quartered along keys so early heads start sooner
        cdts = []
        for c in range(3):
            # cdt[j%128, jt, i] = cdT[c, jt*128 + j%128, i]
            cdt = cdtp.tile([P, 16, IH], F16, tag=f"cdt{c}")
            cdts.append(cdt)
        for jq in range(4):
            for c in range(3):
                nc.gpsimd.dma_start(
                    out=cdts[c][:, jq * 4:(jq + 1) * 4, :],
                    in_=cd_d[c, jq * 512:(jq + 1) * 512, :].rearrange(
                        "(jt p) i -> p jt i", p=P))

        def emit_q(t2):
            for fo in range(4):
                ps = psc.tile([P, IH], F32, tag="ps_s")
                for dc in range(4):
                    nc.tensor.matmul(ps[:, 0:512],
                                     lhsT=wqkv_sb[:, dc, fo * P:(fo + 1) * P],
                                     rhs=xqT[:, t2, dc, :],
                                     start=(dc == 0), stop=(dc == 3))
                nc.any.tensor_copy(out=qT[:, fo, t2 * 512:(t2 + 1) * 512],
                                   in_=ps[:, 0:512])

        def emit_k(t4):
            for fo in range(4):
                ps = psc.tile([P, IH], F32, tag="ps_s")
                for dc in range(4):
                    nc.tensor.matmul(ps[:, 0:512],
                                     lhsT=wqkv_sb[:, dc, D + fo * P:D + (fo + 1) * P],
                                     rhs=xT[:, t4, dc, :],
                                     start=(dc == 0), stop=(dc == 3))
                nc.any.tensor_copy(out=kT[:, fo, t4 * 512:(t4 + 1) * 512], in_=ps[:, 0:512])

        def emit_v(tt):
            ps = psc.tile([P, IH], F32, tag="ps_s")
            for dc in range(4):
                nc.tensor.matmul(ps[:, 0:512],
                                 lhsT=xT[:, tt // 4, dc, (tt % 4) * P:(tt % 4 + 1) * P],
                                 rhs=wqkv_sb[:, dc, 2 * D:3 * D],
                                 start=(dc == 0), stop=(dc == 3))
            nc.any.tensor_copy(out=v_sb[:, tt, :, 0:DH],
                               in_=ps[:, 0:512].rearrange("p (h d) -> p h d", h=HEADS))

        # interleave q/k/v so PE work tracks each transposed chunk's arrival
        emit_q(0)
        emit_k(0)
        for tt in range(4):
            emit_v(tt)
        emit_q(1)
        emit_k(1)
        for tt in range(4, 8):
            emit_v(tt)
        emit_k(2)
        for tt in range(8, 12):
            emit_v(tt)
        emit_k(3)
        for tt in range(12, 16):
            emit_v(tt)
        nc.vector.memset(v_sb[:, :, :, DH:DH + 1], 1.0)
        xtpool.release()
        xqpool.release()
        wqp.release()

        # ---- attention: head-serial, full-width (1024) planes ----
        sbt = ctx.enter_context(tc.tile_pool(name="sbt", bufs=2))
        ptp = ctx.enter_context(tc.tile_pool(name="ptp", bufs=2))
        idp = ctx.enter_context(tc.tile_pool(name="idp", bufs=HEADS))
        normp = ctx.enter_context(tc.tile_pool(name="normp", bufs=2))
        bcp = ctx.enter_context(tc.tile_pool(name="bcp", bufs=1))
        outp = ctx.enter_context(tc.tile_pool(name="outp", bufs=2))
        outT = singles.tile([P, 4, IH], F16)      # [f%128, fo, i]

        # scaled identities for route A, all built upfront on ACT (idle
        # during the qkv phase) so no head ever waits on them
        idhs = {}
        for h in range(HEADS):
            idh = idp.tile([P, 3, P], F16, tag="idh")
            for c in range(3):
                nc.scalar.mul(idh[:, c, :], ident, relw_sb[:, h, c:c + 1])
            idhs[h] = idh

        for h in HEAD_ORDER:
            fo, hp = h // 2, (h % 2) * 64
            idh = idhs[h]
            po = pat.tile([P, IH], F32, tag="po")
            bpend = None     # pending half-filled B exp pair: (jt, b3, pTp)
            for jt in range(16):
                r = ROUTE16[jt]
                ps = psc.tile([P, IH], F32, tag="ps_s")
                pv = []      # (jt, pT-slice) planes ready for the pv matmul
                if r == 'A':
                    # bias first: it depends only on cd + idh, so the PE can
                    # run it while q/k are still being produced (or while
                    # waiting on the exp of older planes)
                    for c in range(3):
                        for half in range(2):
                            sl5 = slice(half * 512, (half + 1) * 512)
                            nc.tensor.matmul(ps[:, sl5], lhsT=idh[:, c, :],
                                             rhs=cdts[c][:, jt, sl5],
                                             start=(c == 0), stop=False,
                                             skip_group_check=True)
                    for half in range(2):
                        sl5 = slice(half * 512, (half + 1) * 512)
                        nc.tensor.matmul(ps[:, sl5],
                                         lhsT=kT[hp:hp + 64, fo, jt * P:(jt + 1) * P],
                                         rhs=qT[hp:hp + 64, fo, sl5],
                                         start=False, stop=(half == 1),
                                         skip_group_check=True)
                    pT = ptp.tile([P, IH], BF, tag="pT")
                    nc.scalar.activation(out=pT, in_=ps, func=EXP,
                                         bias=relb_sb[:, h:h + 1], scale=1.0)
                    pv.append((jt, pT[:, :]))
                else:
                    for half in range(2):
                        sl5 = slice(half * 512, (half + 1) * 512)
                        nc.tensor.matmul(ps[:, sl5],
                                         lhsT=kT[hp:hp + 64, fo, jt * P:(jt + 1) * P],
                                         rhs=qT[hp:hp + 64, fo, sl5],
                                         start=True, stop=True, skip_group_check=True)
                    if bpend is None:
                        b3 = sbt.tile([P, 2, IH], F16, tag="b3")
                        pTp = ptp.tile([P, 2, IH], BF, tag="pTp")
                        side = 0
                    else:
                        _, b3, pTp = bpend
                        side = 1
                    t1 = sbt.tile([P, IH], F16, tag="t1")
                    nc.vector.scalar_tensor_tensor(
                        out=t1, in0=cdts[0][:, jt, :], scalar=relw_sb[:, h, 0:1],
                        in1=ps, op0=MULT, op1=ADD)
                    nc.vector.scalar_tensor_tensor(
                        out=b3[:, side, :], in0=cdts[1][:, jt, :],
                        scalar=relw_sb[:, h, 1:2], in1=t1, op0=MULT, op1=ADD)
                    nc.vector.scalar_tensor_tensor(
                        out=b3[:, side, :], in0=cdts[2][:, jt, :],
                        scalar=relw_sb[:, h, 2:3], in1=b3[:, side, :],
                        op0=MULT, op1=ADD)
                    if side == 0:
                        bpend = (jt, b3, pTp)
                    else:
                        jt0, b3, pTp = bpend
                        bpend = None
                        nc.scalar.activation(out=pTp, in_=b3, func=EXP,
                                             bias=relb_sb[:, h:h + 1], scale=1.0)
                        pv.append((jt0, pTp[:, 0, :]))
                        pv.append((jt, pTp[:, 1, :]))
                for jtp, pTs in pv:
                    for half in range(2):
                        sl5 = slice(half * 512, (half + 1) * 512)
                        nc.tensor.matmul(po[0:DH + 1, sl5], lhsT=v_sb[:, jtp, h, :],
                                         rhs=pTs[:, sl5], start=(jtp == 0),
                                         stop=(jtp == 15), skip_group_check=True)
            # evict po immediately (ACT) so the next head's accumulation can
            # start while the normalizer DMA round-trip is in flight
            o_sb = normp.tile([P, IH], F32, tag="o_sb")
            nc.scalar.copy(out=o_sb[0:DH + 1, :], in_=po[0:DH + 1, :])
            dr = drp.tile([IH], F32, tag="dr")
            nc.sync.dma_start(out=dr[:], in_=o_sb[64:65, :])
            sl = normp.tile([P, 8], F32, tag="sl")
            nc.sync.dma_start(out=sl, in_=dr.rearrange("(p c) -> p c", p=P))
            rs = normp.tile([P, 8], F32, tag="rs")
            nc.vector.reciprocal(out=rs, in_=sl)
            dr2 = drp.tile([IH], F32, tag="dr2")
            nc.sync.dma_start(out=dr2.rearrange("(p c) -> p c", p=P), in_=rs)
            bc = bcp.tile([P, IH], F32, tag="bc")
            nc.sync.dma_start(out=bc[0:64, :], in_=bcast(dr2[:], parts=64))
            if hp == 0:
                nc.vector.tensor_mul(outT[0:64, fo, :], o_sb[0:64, :], bc[0:64, :])
            else:
                tmp = bcp.tile([P, IH], F16, tag="tmp")
                nc.vector.tensor_mul(tmp[0:64, :], o_sb[0:64, :], bc[0:64, :])
                nc.sync.dma_start(out=outT[64:128, fo, :], in_=tmp[0:64, :])

        # ---- output projection ----
        for tt in range(IH // P):
            ps = psc.tile([P, IH], F32, tag="ps_s")
            for fo in range(4):
                nc.tensor.matmul(ps[:, 0:512], lhsT=outT[:, fo, tt * P:(tt + 1) * P],
                                 rhs=wout_sb[:, fo, :], start=(fo == 0), stop=(fo == 3))
            osb = outp.tile([P, D], F32, tag="osb")
            nc.vector.scalar_tensor_tensor(out=osb, in0=ps[:, 0:512], scalar=1.0,
                                           in1=bout_sb, op0=MULT, op1=ADD)
            nc.sync.dma_start(out=out_d[tt * P:(tt + 1) * P, :], in_=osb)

    nc.finalize()
    return nc


_CACHE = {}


def _run(in_maps, trace=False, **kw):
    from concourse.bass_utils import run_bass_kernel_spmd
    nc = _CACHE.get('nc')
    if nc is None:
        nc = build_bass()
        _CACHE['nc'] = nc
    return run_bass_kernel_spmd(nc, in_maps, list(range(NCORES)), trace=trace, **kw)


def make_in_maps(x, centroid_delta, Wqkv, Wout, bout, rel_w, rel_b):
    f32 = lambda a: np.ascontiguousarray(np.asarray(a, dtype=np.float32))
    f16 = lambda a: np.ascontiguousarray(np.asarray(a, dtype=np.float32).astype(np.float16))
    Wqkv = np.asarray(Wqkv, dtype=np.float32).copy()
    Wqkv[:, :D] *= SCALE          # fold the attention scale into the q columns
    x = f16(x)
    centroid_delta = f16(centroid_delta)
    Wqkv = f16(Wqkv)
    Wout = f16(Wout)
    bout = f32(bout)
    rel_w = f32(rel_w)
    rel_b = f32(rel_b)
    in_maps = []
    for cid in range(NCORES):
        b, ihf = cid // 2, cid % 2
        sl = slice(ihf * IH, (ihf + 1) * IH)
        in_maps.append({
            "x": x[b],
            "xq": np.ascontiguousarray(x[b, sl]),
            "cd": np.ascontiguousarray(centroid_delta[b, :, sl, :].transpose(0, 2, 1)),
            "wqkv": Wqkv,
            "wout": Wout,
            "bout": bout,
            "relw": rel_w,
            "relb": rel_b,
        })
    return in_maps


def assemble(results):
    out = np.empty((B, N, D), dtype=np.float32)
    for cid in range(NCORES):
        b, ihf = cid // 2, cid % 2
        out[b, ihf * IH:(ihf + 1) * IH, :] = results[cid]["out"]
    return out


def kernel(x, centroid_delta, Wqkv, Wout, bout, rel_w, rel_b):
    in_maps = make_in_maps(x, centroid_delta, Wqkv, Wout, bout, rel_w, rel_b)
    res = _run(in_maps, trace=False)
    return assemble(res.results)
